# revision 6
# baseline (speedup 1.0000x reference)
"""Trainium2 Bass kernel: 3x3 valid 2D cross-correlation on an 8192x8192 f32 image.

Strategy (8 NeuronCores, pure spatial/data parallel):
  - Row-shard: core i receives input rows [1024*i, 1024*i + 1026) (the 2-row
    halo comes for free since we shard on the host from the full input; the
    tail cores' out-of-range rows are zero-padded and the corresponding
    output rows discarded at gather).
  - Per core: for each 128-input-row tile, the conv is computed as 3
    TensorEngine matmuls accumulating into PSUM:
        out[y, c] = sum_dx (M_dx.T @ X)[y, c+dx]
    where M_dx[k, y] = w[k-y, dx] is a 3-diagonal band matrix built on the
    host from the 3x3 weight. Data moves through the PE in float32r mode
    (full fp32 bits, fast 4-byte streaming path, ~1 cycle/column).
  - PSUM chunks (512 cols) are evacuated to SBUF by the Vector/Scalar
    engines (alternating), then DMA'd to DRAM.
"""

import numpy as np

import concourse.mybir as mybir
from concourse import bacc
from concourse.tile import TileContext
from concourse.bass_utils import run_bass_kernel_spmd

H = W = 8192
KH = KW = 3
N_CORES = 8
OUT_H = H - KH + 1  # 8190
OUT_W = W - KW + 1  # 8190

ROWS_PER_CORE = 1024          # output rows computed per core (core 7: keep 1022)
IN_ROWS_PER_CORE = ROWS_PER_CORE + KH - 1  # 1026
TILE_OUT = 126                # output rows per 128-partition input tile
CHUNK = 512                   # PSUM bank width (fp32)

_NC_CACHE = {}


def _build_program():
    """Build the per-core Bass program (identical on all 8 cores)."""
    nc = bacc.Bacc("TRN2", target_bir_lowering=False, debug=False)
    x = nc.declare_dram_parameter(
        "x", [IN_ROWS_PER_CORE, W], mybir.dt.float32r, isOutput=False
    )
    m = nc.declare_dram_parameter(
        "m", [128, 3 * TILE_OUT], mybir.dt.float32r, isOutput=False
    )
    y = nc.declare_dram_parameter(
        "y", [ROWS_PER_CORE, OUT_W], mybir.dt.float32, isOutput=True
    )

    n_tiles = -(-ROWS_PER_CORE // TILE_OUT)  # 9 (8 full + 1 of 16 rows)
    n_chunks = -(-OUT_W // CHUNK)            # 16 (15 full + 1 of 510)

    HALF_CHUNKS = n_chunks // 2   # 8 chunks per output half-tile
    HALF_W = HALF_CHUNKS * CHUNK  # 4096

    # process runt tile (16 rows) first: tiny load lets PE start early and
    # its expensive-compute/tiny-store profile stays out of the drain phase.
    # Last tile (t7) stores in halves for a fine-grained tail.
    order = [n_tiles - 1] + list(range(n_tiles - 1))

    with TileContext(nc) as tc:
        with (
            tc.tile_pool(name="mp", bufs=1) as mpool,
            tc.tile_pool(name="xp", bufs=3) as xpool,
            tc.tile_pool(name="op", bufs=2) as opool,
            tc.tile_pool(name="oph", bufs=2) as ophalf,
            tc.tile_pool(name="pp", bufs=8, space="PSUM") as ppool,
        ):
            xts = {}

            # runt tile load leads the program so DMA starts immediately
            t_first = order[0]
            xts[t_first] = xpool.tile([128, W], mybir.dt.float32r,
                                      name="xt", tag="xt")
            rf0 = t_first * TILE_OUT
            rf_in = min(TILE_OUT, ROWS_PER_CORE - rf0) + KH - 1
            nc.sync.dma_start(out=xts[t_first][:rf_in],
                              in_=x[rf0:rf0 + rf_in, :])

            mt = mpool.tile([128, 3 * TILE_OUT], mybir.dt.float32r)
            nc.sync.dma_start(out=mt[:], in_=m[:])

            for t in order:
                r0 = t * TILE_OUT
                rows_out = min(TILE_OUT, ROWS_PER_CORE - r0)
                rows_in = rows_out + KH - 1

                if t not in xts:
                    xts[t] = xpool.tile([128, W], mybir.dt.float32r,
                                        name="xt", tag="xt")
                    if t - 1 in xts:
                        # halo rows r0, r0+1 already on-chip as the previous
                        # tile's partitions 126/127: SBUF->SBUF copy saves
                        # HBM read bandwidth
                        nc.sync.dma_start(out=xts[t][0:2, :],
                                          in_=xts[t - 1][126:128, :])
                        nc.sync.dma_start(out=xts[t][2:rows_in],
                                          in_=x[r0 + 2:r0 + rows_in, :])
                    else:
                        nc.sync.dma_start(out=xts[t][:rows_in],
                                          in_=x[r0:r0 + rows_in, :])
                xt = xts[t]

                last = t == order[-1]
                if last:
                    spans = [(h * HALF_W, min(HALF_W, OUT_W - h * HALF_W))
                             for h in range(2)]
                else:
                    spans = [(0, OUT_W)]
                for h0, hw in spans:
                    if last:
                        ot = ophalf.tile([128, HALF_W], mybir.dt.float32,
                                         name="oth", tag="oth")
                    else:
                        ot = opool.tile([128, OUT_W], mybir.dt.float32,
                                        name="ot", tag="ot")
                    nch = -(-hw // CHUNK)
                    for ci in range(nch):
                        c0 = h0 + ci * CHUNK
                        wid = min(CHUNK, h0 + hw - c0)
                        pt = ppool.tile([128, CHUNK], mybir.dt.float32,
                                        name="pt", tag="pt")
                        for dx in range(KW):
                            nc.tensor.matmul(
                                pt[:rows_out, :wid],
                                mt[:rows_in, dx * TILE_OUT:dx * TILE_OUT + rows_out],
                                xt[:rows_in, c0 + dx:c0 + dx + wid],
                                start=(dx == 0),
                                stop=(dx == KW - 1),
                            )
                        dst = ot[:rows_out, ci * CHUNK:ci * CHUNK + wid]
                        if ci % 2 == 0:
                            nc.scalar.copy(out=dst, in_=pt[:rows_out, :wid])
                        else:
                            nc.vector.tensor_copy(out=dst, in_=pt[:rows_out, :wid])
                    nc.scalar.dma_start(out=y[r0:r0 + rows_out, h0:h0 + hw],
                                        in_=ot[:rows_out, :hw])
    nc.compile()
    return nc


def _get_program():
    if "nc" not in _NC_CACHE:
        _NC_CACHE["nc"] = _build_program()
    return _NC_CACHE["nc"]


def _band_matrices(weight: np.ndarray) -> np.ndarray:
    """m[k, dx*126 + y] = w[k-y, dx] for 0 <= k-y < 3."""
    m = np.zeros((128, 3 * TILE_OUT), dtype=np.float32)
    for dx in range(KW):
        for dy in range(KH):
            ys = np.arange(TILE_OUT)
            m[ys + dy, dx * TILE_OUT + ys] = weight[dy, dx]
    return m


def kernel(x: np.ndarray, weight: np.ndarray) -> np.ndarray:
    x = np.ascontiguousarray(np.asarray(x, dtype=np.float32))
    weight = np.asarray(weight, dtype=np.float32)
    assert x.shape == (H, W) and weight.shape == (KH, KW)

    m = _band_matrices(weight)

    # shard rows with halo; zero-pad past the bottom edge
    in_maps = []
    for i in range(N_CORES):
        r0 = i * ROWS_PER_CORE
        r1 = min(r0 + IN_ROWS_PER_CORE, H)
        shard = np.zeros((IN_ROWS_PER_CORE, W), dtype=np.float32)
        shard[: r1 - r0] = x[r0:r1]
        in_maps.append({"x": shard, "m": m})

    nc = _get_program()
    res = run_bass_kernel_spmd(nc, in_maps, core_ids=list(range(N_CORES)))

    out = np.empty((OUT_H, OUT_W), dtype=np.float32)
    for i in range(N_CORES):
        r0 = i * ROWS_PER_CORE
        keep = min(ROWS_PER_CORE, OUT_H - r0)
        out[r0:r0 + keep] = res.results[i]["y"][:keep]
    return out


# revision 10
# speedup vs baseline: 1.2207x; 1.2207x over previous
"""Trainium2 Bass kernel: 3x3 valid 2D cross-correlation on an 8192x8192 f32 image.

Strategy (8 NeuronCores, pure spatial/data parallel):
  - Row-shard on the host: core i receives input rows [1024*i, 1024*i + 1026)
    (the 2-row halo is free since we shard from the full input; rows past the
    bottom edge are zero-padded and the corresponding outputs discarded).
  - Per core, raw-bass pipeline (manual semaphores, no framework preamble):
    9 row-tiles (128 input partitions -> 126 output rows; last tile 18->16).
    For each tile, 16 column chunks of 512; per chunk 3 TensorEngine matmuls
    accumulate into a PSUM bank:
        out[y, c] = sum_dx (M_dx.T @ X)[y, c+dx]
    where M_dx[k, y] = w[k-y, dx] is a 3-diagonal band matrix built on the
    host from the 3x3 weight. Matmul operands use float32r (fp32 bits on the
    fast 4-byte PE streaming path, ~1 cycle/column, ~1e-4 rel err).
  - ScalarE copies even chunks PSUM->SBUF, VectorE odd chunks; SP ring does
    x loads, ACT ring does y stores (full-width rows; the last two tiles
    store in halves to drain the tail at fine granularity).
  - The kernel is HBM-bandwidth-bound: DMA measured busy end-to-end at
    ~360 GB/s per core with no idle gaps.
"""

import numpy as np

import concourse.bass as bass
import concourse.mybir as mybir
from concourse.bass_utils import run_bass_kernel_spmd

H = W = 8192
KH = KW = 3
N_CORES = 8
OUT_H = H - KH + 1  # 8190
OUT_W = W - KW + 1  # 8190

ROWS_PER_CORE = 1024          # output rows per core (core 7: keep 1022)
IN_ROWS_PER_CORE = ROWS_PER_CORE + KH - 1  # 1026
TILE_OUT = 126                # output rows per 128-partition input tile
CHUNK = 512                   # PSUM bank width (fp32)
N_TILES = 9
N_CHUNKS = 16
HALF_W = 4096
XBUFS = 3
OBUFS = 2

_NC_CACHE = {}


def _build_program():
    nc = bass.Bass("TRN2", target_bir_lowering=False, debug=False)
    x = nc.declare_dram_parameter(
        "x", [IN_ROWS_PER_CORE, W], mybir.dt.float32r, isOutput=False
    )
    m = nc.declare_dram_parameter(
        "m", [128, 3 * TILE_OUT], mybir.dt.float32r, isOutput=False
    )
    y = nc.declare_dram_parameter(
        "y", [ROWS_PER_CORE, OUT_W], mybir.dt.float32, isOutput=True
    )

    xb = [nc.alloc_sbuf_tensor(f"xb{i}", [128, W], mybir.dt.float32r).ap()
          for i in range(XBUFS)]
    ob = [nc.alloc_sbuf_tensor(f"ob{i}", [128, OUT_W], mybir.dt.float32).ap()
          for i in range(OBUFS)]
    mt = nc.alloc_sbuf_tensor("mt", [128, 3 * TILE_OUT], mybir.dt.float32r).ap()
    pb = [nc.alloc_psum_tensor(f"pb{i}", [128, CHUNK], mybir.dt.float32).ap()
          for i in range(8)]

    sx = [nc.alloc_semaphore(f"sx{t}") for t in range(N_TILES)]
    sm = nc.alloc_semaphore("sm")
    s_mm = nc.alloc_semaphore("s_mm")
    s_cpA = nc.alloc_semaphore("s_cpA")
    s_cpD = nc.alloc_semaphore("s_cpD")
    sst = [nc.alloc_semaphore(f"sst{j}") for j in range(N_TILES)]

    def rows_of(t):
        rows_out = min(TILE_OUT, ROWS_PER_CORE - t * TILE_OUT)
        return rows_out, rows_out + KH - 1

    with nc.Block() as block:

        @block.sync
        def _(sync):
            for t in range(N_TILES):
                r0 = t * TILE_OUT
                _, rows_in = rows_of(t)
                if t == 1:
                    sync.dma_start(out=mt, in_=m[:]).then_inc(sm, 16)
                if t >= XBUFS:
                    # x slot reuse: previous tile in this slot fully consumed
                    sync.wait_ge(s_mm, 16 * (t - XBUFS + 1))
                sync.dma_start(
                    out=xb[t % XBUFS][:rows_in], in_=x[r0:r0 + rows_in, :]
                ).then_inc(sx[t], 16)
            for t in range(N_TILES):
                sync.wait_ge(sst[t], 32 if t >= N_TILES - 2 else 16)

        @block.tensor
        def _(tensor):
            tensor.wait_ge(sm, 16)
            for t in range(N_TILES):
                rows_out, rows_in = rows_of(t)
                tensor.wait_ge(sx[t], 16)
                for k in range(N_CHUNKS):
                    g = t * N_CHUNKS + k
                    b = g % 8
                    if g >= 8:
                        # PSUM bank b free once chunk g-8's copy retired
                        tp, kp = divmod(g - 8, N_CHUNKS)
                        if kp % 2 == 0:
                            tensor.wait_ge(s_cpA, 8 * tp + kp // 2 + 1)
                        else:
                            tensor.wait_ge(s_cpD, 8 * tp + (kp - 1) // 2 + 1)
                    c0 = k * CHUNK
                    wid = min(CHUNK, OUT_W - c0)
                    for dx in range(KW):
                        ins = nc.tensor.matmul(
                            pb[b][:rows_out, :wid],
                            mt[:rows_in, dx * TILE_OUT:dx * TILE_OUT + rows_out],
                            xb[t % XBUFS][:rows_in, c0 + dx:c0 + dx + wid],
                            start=(dx == 0),
                            stop=(dx == KW - 1),
                        )
                        if dx == KW - 1:
                            ins.then_inc(s_mm, 1)

        @block.scalar
        def _(scalar):
            for t in range(N_TILES):
                rows_out, _ = rows_of(t)
                r0 = t * TILE_OUT
                if t >= OBUFS:
                    scalar.wait_ge(sst[t - OBUFS], 16)

                def act_copy(k):
                    g = t * N_CHUNKS + k
                    c0 = k * CHUNK
                    wid = min(CHUNK, OUT_W - c0)
                    scalar.wait_ge(s_mm, g + 1)
                    nc.scalar.copy(
                        out=ob[t % OBUFS][:rows_out, c0:c0 + wid],
                        in_=pb[g % 8][:rows_out, :wid],
                    ).then_inc(s_cpA, 1)

                if t < N_TILES - 2:
                    for k in range(0, N_CHUNKS, 2):
                        act_copy(k)
                    scalar.wait_ge(s_cpA, 8 * (t + 1))
                    scalar.wait_ge(s_cpD, 8 * (t + 1))
                    scalar.dma_start(
                        out=y[r0:r0 + rows_out, :],
                        in_=ob[t % OBUFS][:rows_out, :],
                    ).then_inc(sst[t], 16)
                else:
                    # drain tiles: store halves as soon as each is copied
                    for k in range(0, N_CHUNKS // 2, 2):
                        act_copy(k)
                    scalar.wait_ge(s_cpA, 8 * t + 4)
                    scalar.wait_ge(s_cpD, 8 * t + 4)
                    scalar.dma_start(
                        out=y[r0:r0 + rows_out, :HALF_W],
                        in_=ob[t % OBUFS][:rows_out, :HALF_W],
                    ).then_inc(sst[t], 16)
                    for k in range(N_CHUNKS // 2, N_CHUNKS, 2):
                        act_copy(k)
                    scalar.wait_ge(s_cpA, 8 * (t + 1))
                    scalar.wait_ge(s_cpD, 8 * (t + 1))
                    scalar.dma_start(
                        out=y[r0:r0 + rows_out, HALF_W:],
                        in_=ob[t % OBUFS][:rows_out, HALF_W:OUT_W],
                    ).then_inc(sst[t], 16)

        @block.vector
        def _(vector):
            for t in range(N_TILES):
                rows_out, _ = rows_of(t)
                if t >= OBUFS:
                    vector.wait_ge(sst[t - OBUFS], 16)
                for k in range(1, N_CHUNKS, 2):
                    g = t * N_CHUNKS + k
                    c0 = k * CHUNK
                    wid = min(CHUNK, OUT_W - c0)
                    vector.wait_ge(s_mm, g + 1)
                    nc.vector.tensor_copy(
                        out=ob[t % OBUFS][:rows_out, c0:c0 + wid],
                        in_=pb[g % 8][:rows_out, :wid],
                    ).then_inc(s_cpD, 1)

    return nc


def _get_program():
    if "nc" not in _NC_CACHE:
        _NC_CACHE["nc"] = _build_program()
    return _NC_CACHE["nc"]


def _band_matrices(weight: np.ndarray) -> np.ndarray:
    """m[k, dx*126 + y] = w[k-y, dx] for 0 <= k-y < 3."""
    mm = np.zeros((128, 3 * TILE_OUT), dtype=np.float32)
    for dx in range(KW):
        for dy in range(KH):
            ys = np.arange(TILE_OUT)
            mm[ys + dy, dx * TILE_OUT + ys] = weight[dy, dx]
    return mm


def _in_maps(x, weight):
    mmat = _band_matrices(weight)
    maps = []
    for i in range(N_CORES):
        r0 = i * ROWS_PER_CORE
        r1 = min(r0 + IN_ROWS_PER_CORE, H)
        shard = np.zeros((IN_ROWS_PER_CORE, W), dtype=np.float32)
        shard[: r1 - r0] = x[r0:r1]
        maps.append({"x": shard, "m": mmat})
    return maps


def kernel(x: np.ndarray, weight: np.ndarray) -> np.ndarray:
    x = np.ascontiguousarray(np.asarray(x, dtype=np.float32))
    weight = np.asarray(weight, dtype=np.float32)
    assert x.shape == (H, W) and weight.shape == (KH, KW)

    nc = _get_program()
    res = run_bass_kernel_spmd(nc, _in_maps(x, weight),
                               core_ids=list(range(N_CORES)))

    out = np.empty((OUT_H, OUT_W), dtype=np.float32)
    for i in range(N_CORES):
        r0 = i * ROWS_PER_CORE
        keep = min(ROWS_PER_CORE, OUT_H - r0)
        out[r0:r0 + keep] = res.results[i]["y"][:keep]
    return out
